# revision 53
# baseline (speedup 1.0000x reference)
"""Trainium2 Bass kernel for nn_AttentionHead (causal single-head attention
with input projections), data-parallel over the batch dim on 8 NeuronCores.

Per-core computation (batch b):
  qh = q[b] @ Wq ; kh = k[b] @ Wk ; vh = v[b] @ Wv        [2048, 64]
  scores = (qh @ kh^T) * 8, causal-masked, softmax over s
  out[b] = softmax(scores) @ vh                            [2048, 64]

Design ("two-pass QK, transpose-free"), driven by the TimelineSim cost
model (DMA stream floor ~38us/core; matmul cost = out-cols x 0.42ns;
Ldweights free; engines have in-order queues, so emission order is the
schedule):

  - Inputs ship fp16, e-major ([128, 8, L]) so projections contract on
    partitions. Wq is host-scaled by -8: QK directly gives n = -8*scores,
    so the softmax bias is b = row-min(n) and exp uses scale=-1.
  - Contraction-row layout (engine partition starts must be 32-aligned):
    qsplit rows 0-63 = r12(hi product), 64-95 = r12(lo[0:32]), row 96 =
    -b, 97+ = zero. kdup rows 0-63 = kcomb (11-bit Wk, single), 64-95 =
    kcomb[0:32], row 96 = 1.0. Row 96 x row 96 folds (n - b) into the
    pass-B matmul itself.
  - Pass A (bias): QK in [l, s] orientation; per (tile, chunk) a negated
    min-reduce chains through an elementwise max into bm; the diagonal
    block is masked by adding a +1e30 triangle first. -min is transposed
    to a row on the PE (fp16, permutation identity) and written into
    qsplit row 96 by the Act engine.
  - Pass B: QK re-computed transposed ([s, l]: lhsT = kdup s-block, rhs =
    qsplit l-chunk, fp32r); exp (scale=-1) writes E^T fp16 straight to
    its final SBUF home -- no PE transposes, no PSUM->SBUF E copies.
  - v projection is flipped (stationary vT s-block, streaming Wv):
    half the PE cycles, vh lands directly in [s, d] orientation, and a
    ones-column makes the AV matmul accumulate Z for free.
  - AV: lhsT = E^T block, rhs = vones [s, 65]; out[l, 64]/out[l, 0:64]
    normalizes via DVE reciprocal + scalar mul. Output is fp16 in a
    [128, 16, 64] permuted layout (contiguous 512B DMA runs), unpermuted
    and cast on the host.
  - Schedule: software pipeline; iteration it emits proj(it), pass-A of
    chunk it, and pass-B of strip it-1 interleaved; all q/k chunks stream
    before any v so the bias-critical q3/k3 land ~8us earlier; the v
    projections, AV, normalize and flushes are deferred to a tail block so
    they never sit ahead of the bias chain in the DVE queue. Outputs
    flush on the Act queue; per-half PSUM tiles let qsplit copies start
    at half-chunk arrival.
  - Numerics: ~13-bit q, 12-bit k, fp16 E with exact row-min bias
    (measured rel err ~3.4e-3 vs fp32 reference; bias precision cancels
    in the normalization). tensor_tensor_reduce passes CoreSim but fails
    on real hardware, so the shipped config uses plain chained reduces
    (amin_mode=red); gpsimd affine_select and fp32 PE transpose were
    device-validated and are enabled.
"""
import sys

if "/opt/trn_rl_repo" not in sys.path:
    sys.path.insert(0, "/opt/trn_rl_repo")

import numpy as np

N_CORES = 8
NB, L, S, E, D = 8, 2048, 2048, 1024, 64
P = 128
ECH = E // P          # 8 e-chunks
LCH = 4               # l/s chunks of 512
NLT = L // P          # 16 l-tiles
NST = S // P          # 16 s-tiles
CHUNK = 512
H = CHUNK // 2        # DMA half-chunk (256 cols)

# const blob column offsets (fp16 cols)
OFF_WQ = 0
OFF_WK = 1024
BLOB1_COLS = 1536
OFF_WV = 0
OFF_IDN = 512     # +I, f32 [128,128] (256 f16 cols)
OFF_MM = 768      # master mask, f32 [128,640] (1280 f16 cols)
OFF_ID16 = 2048   # +I, f16 [128,128]
OFF_DMT = 2176    # pass-B diag mask (l<s), f32 [128,128] (256 f16 cols)
BLOB2_COLS = 2432

_PROGRAM = None

# schedule/engine-assignment knobs (swept via TimelineSim)
CFG = {
    "pre_frac": 0,         # quarters of strip(it-1) emitted before proj(it)
    "qsplit_eng": "dve",   # qsplit copy engine: dve|act
    "kcomb_eng": "act",    # kcomb copy engine: dve|act
    "tsmul_eng": "dve",    # normalize mul engine: act|dve
    "vones_eng": "dve",    # vones copy engine: dve|act
    "mask_mode": "affine",  # affine|dvett: how pass-B diag is masked
    "bt16": False,          # bias transpose in fp16 (device-proven) vs fp32
    "amin_mode": "red",    # ttr|red: fused TTR chain vs plain reduces
    "brow_eng": "act",     # act|dve: engine for the bias-row copy
}


def _build_program():
    import concourse.bacc as bacc
    import concourse.mybir as mybir
    import concourse.tile as tile
    from concourse.bass import ds

    F32 = mybir.dt.float32
    F16 = mybir.dt.float16
    F32R = mybir.dt.float32r
    Exp = mybir.ActivationFunctionType.Exp
    AX = mybir.AxisListType.X
    MIN = mybir.AluOpType.min

    nc = bacc.Bacc(None, target_bir_lowering=False)

    kT = nc.declare_dram_parameter("kT", [P, ECH, S], F16, isOutput=False)
    qT = nc.declare_dram_parameter("qT", [P, ECH, L], F16, isOutput=False)
    vT = nc.declare_dram_parameter("vT", [P, ECH, S], F16, isOutput=False)
    blob_d = nc.declare_dram_parameter("blob", [P, BLOB1_COLS], F16, isOutput=False)
    blob2_d = nc.declare_dram_parameter("blob2", [P, BLOB2_COLS], F16, isOutput=False)
    out_d = nc.declare_dram_parameter("out", [P, NLT, D], F16, isOutput=True)

    with tile.TileContext(nc) as tc:
        with (
            tc.tile_pool(name="consts", bufs=1) as consts,
            tc.tile_pool(name="persist", bufs=1) as persist,
            tc.tile_pool(name="xstream", bufs=6) as xstream,
            tc.tile_pool(name="work", bufs=4) as work,
            tc.tile_pool(name="obuf", bufs=2) as obuf,
            tc.tile_pool(name="psA", bufs=2, space="PSUM") as psA,
            tc.tile_pool(name="psK", bufs=1, space="PSUM") as psK,
            tc.tile_pool(name="psB", bufs=3, space="PSUM") as psB,
            tc.tile_pool(name="psS", bufs=2, space="PSUM") as psS,
        ):
            # ---- constants (two DMAs: projection weights first) ----
            blob = consts.tile([P, BLOB1_COLS], F16, tag="blob")
            nc.sync.dma_start(out=blob, in_=blob_d[:])
            blob2 = consts.tile([P, BLOB2_COLS], F16, tag="blob2")
            wq2 = blob[:, ds(OFF_WQ, 1024)].rearrange("p (c j) -> p c j", c=ECH)
            wk1 = blob[:, ds(OFF_WK, 512)].rearrange("p (c j) -> p c j", c=ECH)
            wv = blob2[:, ds(OFF_WV, 512)].rearrange("p (c d) -> p c d", c=ECH)
            idneg = blob2[:, ds(OFF_IDN, 2 * P)].bitcast(F32)
            mmask = blob2[:, ds(OFF_MM, 2 * 640)].bitcast(F32)
            id16 = blob2[:, ds(OFF_ID16, P)]
            dmaskT = blob2[:, ds(OFF_DMT, 2 * P)].bitcast(F32)

            # ---- persistent tensors ----
            # qsplit rows: 0-63 r12(-8*qh hi), 64-95 r12(lo[0:32]),
            # 96 = -b (engine partition starts must be 32-aligned), 97+ = 0
            qsplit = persist.tile([P, L], F32R, tag="qsp", name="qsp")
            # kdup[c] rows: 0-63 kcomb, 64-95 kcomb[0:32], 96 = 1.0, 97+ = 0
            kdup = [persist.tile([P, CHUNK], F32R, tag=f"kd{c}", name=f"kd{c}")
                    for c in range(LCH)]
            # E^T[s, l] per s-block j, fp16
            et = persist.tile([P, NST, L], F16, tag="et", name="et")
            # vones[:, j, :]: cols 0-63 = vh rows, col 64 = 1.0
            vones = persist.tile([P, NST, D + 1], F16, tag="vo", name="vo")
            nc.gpsimd.memset(vones[:, :, D : D + 1], 1.0)
            nc.gpsimd.memset(qsplit[96:P, :].bitcast(F32), 0.0)
            for c in range(LCH):
                nc.gpsimd.memset(kdup[c][96:P, :].bitcast(F32), 0.0)
                nc.vector.memset(kdup[c][96:97, :].bitcast(F32), 1.0)

            def proj_k(lc):
                kt = xstream.tile([P, ECH, CHUNK], F16, tag="xs", name="kt")
                ps = psK.tile([P, CHUNK], F32, tag="k")
                for h in range(2):
                    hs = ds(h * H, H)
                    nc.sync.dma_start(
                        out=kt[:, :, hs], in_=kT[:, :, ds(lc * CHUNK + h * H, H)]
                    )
                    for c in range(ECH):
                        nc.tensor.matmul(
                            ps[:D, hs], wk1[:, c, :], kt[:, c, hs],
                            start=(c == 0), stop=(c == ECH - 1),
                        )
                return ps

            def kcomb(lc, ps):
                kd = kdup[lc]
                if CFG["kcomb_eng"] == "dve":
                    nc.vector.tensor_copy(out=kd[:D, :], in_=ps[:D, :])
                else:
                    nc.scalar.copy(out=kd[:D, :], in_=ps[:D, :])
                nc.gpsimd.tensor_copy(out=kd[D:96, :], in_=kd[:32, :])

            def proj_q(lc):
                qt = xstream.tile([P, ECH, CHUNK], F16, tag="xs", name="qt")
                for h in range(2):
                    hs = ds(h * H, H)
                    nc.sync.dma_start(
                        out=qt[:, :, hs], in_=qT[:, :, ds(lc * CHUNK + h * H, H)]
                    )
                    psf = psA.tile([P, CHUNK], F32, tag="a", name="psf")
                    ps = psf[:, :H]
                    for c in range(ECH):
                        nc.tensor.matmul(
                            ps, wq2[:, c, :], qt[:, c, hs],
                            start=(c == 0), stop=(c == ECH - 1),
                        )
                    if CFG["qsplit_eng"] == "dve":
                        nc.vector.tensor_copy(
                            out=qsplit[:96, ds(lc * CHUNK + h * H, H)],
                            in_=ps[:96, :],
                        )
                    else:
                        nc.scalar.copy(
                            out=qsplit[:96, ds(lc * CHUNK + h * H, H)],
                            in_=ps[:96, :],
                        )

            bms = {}

            def amin(i, ps, n, moff, first):
                """Chained masked row-min: bm = max(bm, max(-(ps + mmask)))."""
                bm = bms[i]
                if CFG["amin_mode"] == "ttr":
                    scr = work.tile([P, CHUNK], F16, tag="scr")
                    nc.vector.tensor_tensor_reduce(
                        out=scr[:, :n], in0=ps[:, :n], in1=mmask[:, ds(moff, n)],
                        scale=-1.0, scalar=-3.0e38 if first else bm,
                        op0=mybir.AluOpType.add, op1=mybir.AluOpType.max,
                        accum_out=bm,
                    )
                    return
                if CFG["amin_mode"] == "ttr2":
                    scr = work.tile([P, CHUNK], F16, tag="scr")
                    nxt = work.tile([P, 1], F32, tag="bm2", name="nxt")
                    nc.vector.tensor_tensor_reduce(
                        out=scr[:, :n], in0=ps[:, :n], in1=mmask[:, ds(moff, n)],
                        scale=-1.0, scalar=-3.0e38 if first else bm,
                        op0=mybir.AluOpType.add, op1=mybir.AluOpType.max,
                        accum_out=nxt,
                    )
                    bms[i] = nxt
                    return
                # plain-reduce path: mask the diag block via TT add, then
                # negated min-reduce chained through elementwise max
                if moff > 0:
                    jo = n - P
                    nc.vector.tensor_add(
                        out=ps[:, ds(jo, P)], in0=ps[:, ds(jo, P)],
                        in1=mmask[:, ds(CHUNK, P)],
                    )
                if first:
                    m1 = bm
                else:
                    m1 = work.tile([P, 1], F32, tag="m1", name="m1")
                nc.vector.tensor_reduce(
                    out=m1, in_=ps[:, :n], axis=AX, op=MIN, negate=True
                )
                if not first:
                    nc.vector.tensor_tensor(
                        out=bm, in0=bm, in1=m1, op=mybir.AluOpType.max
                    )

            def pass_a_nondiag(i):
                """Non-diag row-min chunks for l-tile i (needs q(lc), k(<lc))."""
                lc = i // 4
                bms[i] = work.tile([P, 1], F32, tag="bm", name="bm")
                for c2 in range(lc):
                    ps = psA.tile([P, CHUNK], F32, tag="a")
                    nc.tensor.matmul(
                        ps, qsplit[:, ds(i * P, P)], kdup[c2],
                        start=True, stop=True,
                    )
                    amin(i, ps, CHUNK, 0, c2 == 0)

            def pass_a_diag(i):
                """Diag chunk + bias write for l-tile i (needs k(lc))."""
                lc, k = i // 4, i % 4
                n = (k + 1) * P
                ps = psA.tile([P, CHUNK], F32, tag="a")
                nc.tensor.matmul(
                    ps[:, : max(256, n)], qsplit[:, ds(i * P, P)],
                    kdup[lc][:, : max(256, n)],
                    start=True, stop=True,
                )
                amin(i, ps, n, CHUNK - k * P, lc == 0)
                bm = bms.pop(i)
                pss = psS.tile([P, CHUNK], F32, tag="s", name="pss")
                if CFG["bt16"]:
                    bm16 = work.tile([P, 1], F16, tag="bm16")
                    nc.vector.tensor_copy(out=bm16, in_=bm)
                    pst = pss[0:1, 0:D].bitcast(F16)
                    nc.tensor.transpose(pst, bm16, id16)
                else:
                    pst = pss[0:1, 0:P]
                    nc.tensor.transpose(pst, bm, idneg)
                if CFG["brow_eng"] == "dve":
                    nc.vector.tensor_copy(
                        out=qsplit[96:97, ds(i * P, P)], in_=pst
                    )
                else:
                    nc.scalar.copy(out=qsplit[96:97, ds(i * P, P)], in_=pst)

            def pass_b_tile(lc, j, w0=0, w1=CHUNK, sel=True):
                """n^T - b for s-block j vs cols [w0,w1) of l-chunk lc."""
                jb = j % 4
                c0 = max(w0, 0 if j < 4 * lc else min(jb * P, CHUNK - 2 * P))
                c0 = min(c0, w1 - 2 * P)
                n = w1 - c0
                ps = psB.tile([P, CHUNK], F32, tag="b")
                nc.tensor.matmul(
                    ps[:, c0 : c0 + n], kdup[j // 4][:, ds(jb * P, P)],
                    qsplit[:, ds(lc * CHUNK + c0, n)],
                    start=True, stop=True,
                )
                if sel and j >= 4 * lc and CFG["mask_mode"] == "dvett":
                    jo = j * P - lc * CHUNK
                    nc.vector.tensor_add(
                        out=ps[:, ds(jo, P)], in0=ps[:, ds(jo, P)], in1=dmaskT
                    )
                c0e = max(c0, 0 if j < 4 * lc else min(jb * P, w1 - P))
                ne = w1 - c0e
                nc.scalar.activation(
                    out=et[:, j, ds(lc * CHUNK + c0e, ne)],
                    in_=ps[:, c0e : c0e + ne],
                    func=Exp, bias=0.0, scale=-1.0,
                )
                if sel and j >= 4 * lc and CFG["mask_mode"] == "affine":
                    # zero E where l < s in the diagonal block
                    nc.gpsimd.affine_select(
                        out=et[:, j, ds(j * P, P)],
                        in_=et[:, j, ds(j * P, P)],
                        pattern=[[1, P]], base=0, channel_multiplier=-1,
                        compare_op=mybir.AluOpType.is_ge, fill=0.0,
                    )

            def dma_v(lc):
                vt = xstream.tile([P, ECH, CHUNK], F16, tag="xs", name="vt")
                for h in range(2):
                    hs = ds(h * H, H)
                    nc.sync.dma_start(
                        out=vt[:, :, hs], in_=vT[:, :, ds(lc * CHUNK + h * H, H)]
                    )
                return vt

            def proj_v(lc, vt, eng):
                for sb in range(4):
                    j = lc * 4 + sb
                    psv = psS.tile([P, CHUNK], F32, tag="s", name="psv")
                    ps = psv[:, :D]
                    for c in range(ECH):
                        nc.tensor.matmul(
                            ps, vt[:, c, ds(sb * P, P)], wv[:, c, :],
                            start=(c == 0), stop=(c == ECH - 1),
                        )
                    if eng == "dve":
                        nc.vector.tensor_copy(out=vones[:, j, :D], in_=ps)
                    else:
                        nc.scalar.copy(out=vones[:, j, :D], in_=ps)

            def av(lc, ks=range(4)):
                ob = obs.setdefault(lc, obuf.tile([P, 4, D], F16, tag="ob", name="ob"))
                for k in ks:
                    i = lc * 4 + k
                    psp = psS.tile([P, CHUNK], F32, tag="s", name="psp")
                    pav = psp[:, : D + 1]
                    for j in range(i + 1):
                        nc.tensor.matmul(
                            pav, et[:, j, ds(i * P, P)], vones[:, j, :],
                            start=(j == 0), stop=(j == i),
                        )
                    zi = work.tile([P, 1], F32, tag="zi")
                    nc.vector.reciprocal(zi, pav[:, D : D + 1])
                    if CFG["tsmul_eng"] == "act":
                        nc.scalar.mul(ob[:, k, :], pav[:, :D], zi)
                    else:
                        nc.vector.tensor_scalar_mul(ob[:, k, :], pav[:, :D], zi)

            def flush_out(lc):
                nc.scalar.dma_start(
                    out=out_d[:, ds(lc * 4, 4), :], in_=obs.pop(lc)
                )

            # Software pipeline over 5 iterations: iteration `it` emits
            # projections(it), passA(it) interleaved tile-by-tile with
            # passB(it-1) (so DVE reduce work and Act exp work from adjacent
            # chunks overlap despite in-order engine queues), then
            # proj_v(it-1) and av(it-1). v DMAs for chunks 2,3 are issued
            # after k3/q3 so the bias-critical tail data arrives ~4us sooner.
            obs, vts = {}, {}
            fl = LCH - 1
            for it in range(LCH):
                # strip(it-1): pre-proj portion keeps PE fed while q(it)/k(it)
                # stream; the rest interleaves with the pass_a slots
                nb = 4 * it
                pre = nb * CFG["pre_frac"] // 4
                done = 0
                while done < pre:
                    pass_b_tile(it - 1, done)
                    done += 1
                proj_q(it)
                psk = proj_k(it)
                if it == 0:
                    nc.sync.dma_start(out=blob2, in_=blob2_d[:])
                if it == fl:
                    for c in range(LCH):
                        vts[c] = dma_v(c)
                slots = 5
                for sl in range(slots):
                    if sl < 4:
                        pass_a_nondiag(it * 4 + sl)
                    else:
                        kcomb(it, psk)
                    want = pre + (nb - pre) * (sl + 1) // slots
                    while done < want:
                        pass_b_tile(it - 1, done)
                        done += 1
                for k in range(4):
                    pass_a_diag(it * 4 + k)
            # tail: chunks 0-2's v-projections/AV first (ready as v streams
            # arrive; must not sit behind strip-3's b15-gated matmuls), then
            # the final strip, then av(3)
            for lc in range(LCH - 1):
                proj_v(lc, vts.pop(lc), "act")
                av(lc)
                flush_out(lc)
            for j in range(4 * fl + 4):
                pass_b_tile(fl, j)
            proj_v(fl, vts.pop(fl), "dve")
            av(fl, ks=(0, 1))
            ob3 = obs[fl]
            nc.scalar.dma_start(out=out_d[:, ds(fl * 4, 2), :], in_=ob3[:, :2, :])
            av(fl, ks=(2, 3))
            nc.scalar.dma_start(
                out=out_d[:, ds(fl * 4 + 2, 2), :], in_=ob3[:, 2:, :]
            )

    nc.finalize()
    return nc


def _get_program():
    global _PROGRAM
    if _PROGRAM is None:
        _PROGRAM = _build_program()
    return _PROGRAM


def make_in_maps(q, k, v, Wq, Wk, Wv):
    """Host-side sharding + layout prep. Returns one input map per core."""
    def w_split(W):
        W = np.asarray(W, dtype=np.float32)
        hi = W.astype(np.float16)
        lo = (W - hi.astype(np.float32)).astype(np.float16)
        # [E, 2D] -> [ECH, P, 2D] -> [P, ECH*2D]
        return (
            np.concatenate([hi, lo], axis=1).reshape(ECH, P, 2 * D)
            .transpose(1, 0, 2).reshape(P, ECH * 2 * D)
        )

    blob = np.zeros((P, BLOB1_COLS), dtype=np.float16)
    blob2 = np.zeros((P, BLOB2_COLS), dtype=np.float16)
    blob[:, OFF_WQ : OFF_WQ + 1024] = w_split(np.asarray(Wq, np.float32) * np.float32(-8.0))
    blob[:, OFF_WK : OFF_WK + 512] = (
        np.asarray(Wk, np.float32).astype(np.float16)
        .reshape(ECH, P, D).transpose(1, 0, 2).reshape(P, ECH * D)
    )
    blob2[:, OFF_WV : OFF_WV + 512] = (
        np.asarray(Wv, np.float32).astype(np.float16)
        .reshape(ECH, P, D).transpose(1, 0, 2).reshape(P, ECH * D)
    )
    blob2[:, OFF_IDN : OFF_IDN + 2 * P] = (
        np.eye(P, dtype=np.float32)
    ).view(np.float16)
    mm = np.zeros((P, 640), dtype=np.float32)
    mm[:, 512:] = np.where(
        np.arange(P)[None, :] > np.arange(P)[:, None], np.float32(1e30), np.float32(0)
    )
    blob2[:, OFF_MM : OFF_MM + 2 * 640] = mm.view(np.float16)
    blob2[:, OFF_ID16 : OFF_ID16 + P] = np.eye(P, dtype=np.float16)
    dmt = np.where(
        np.arange(P)[None, :] < np.arange(P)[:, None], np.float32(1e30), np.float32(0)
    ).astype(np.float32)
    blob2[:, OFF_DMT : OFF_DMT + 2 * P] = dmt.view(np.float16)

    in_maps = []
    for b in range(N_CORES):
        def xt(x):
            return np.ascontiguousarray(
                np.asarray(x, dtype=np.float32).T
                .reshape(ECH, P, -1).transpose(1, 0, 2)
            ).astype(np.float16)

        in_maps.append({
            "qT": xt(q[b]), "kT": xt(k[b]), "vT": xt(v[b]),
            "blob": blob, "blob2": blob2,
        })
    return in_maps


def kernel(q, k, v, Wq, Wk, Wv, attn_mask=None):
    from concourse.bass_utils import run_bass_kernel_spmd

    nc = _get_program()
    in_maps = make_in_maps(q, k, v, Wq, Wk, Wv)
    res = run_bass_kernel_spmd(nc, in_maps, core_ids=list(range(N_CORES)))
    out = np.stack(
        [
            res.results[b]["out"].transpose(1, 0, 2).reshape(L, D)
            for b in range(N_CORES)
        ],
        axis=0,
    )
    return out.astype(np.float32)
